# revision 38
# baseline (speedup 1.0000x reference)
"""Causal self-attention (B=2, T=2048, C=1024, H=16, D=64) on 8 trn2 NeuronCores.

Sharding: tensor-parallel over heads. Each core owns 2 heads:
  - W_attn columns for its heads (q/k/v slices)  -> per-core qkv width 384
  - W_proj rows for its heads                    -> per-core partial output
Attention is embarrassingly parallel over (B, head). Each core returns a
partial [B*T, C] output (y_local @ W_proj_shard) in fp16; the host sums
the 8 partials in float64 (the row-parallel unshard reduction).

Design notes (all matmul operands fp16, accumulation fp32 in PSUM):
  0. HOST pre-staging: x is cast to fp16 AND transposed on the host, so the
     device reads x^T [C, BT] with plain contiguous DMAs -- no on-device
     f32->f16 cast (was 34us DVE) and no xbar DMA-transposes of x (was 65us
     of DMA fabric). W_attn/W_proj are host-cast to fp16 too.
  1. qkvT [384, 4096] = W^T x^T accumulated over 8 C-tiles into PSUM,
     evacuated (DVE) to a combined fp16 qkv^T buffer. v^T is re-transposed
     by 2 small xbar DMAs/chunk into natural V_aug tiles
     [Tk 128, v_h0|1|pad|v_h1|1|pad] (stride 160, 32B-aligned dests); each
     head's stationary operand is a contiguous 65-column slice whose
     ones-column computes the softmax denominators for free.
  2. Scores TRANSPOSED: S^T [Tk 128, Tq 512] = k^T.T @ q^T (K=64), TWO
     key-tiles per 2-bank PSUM tile (double-buffered) so the QK->exp->AV
     chain pipelines. Causality exploited at 128 granularity: for diagonal
     key-tiles the QK/exp/AV all start at query column 128*(j-4i) (the
     block-triangular raggedness), and the 128x128 triangular blocks are
     masked by a DVE multiply with a constant lower-tri mask (no GPSIMD
     affine_select in the inner loop). Softmax max-subtraction is skipped
     (logits ~N(0,1) after the 1/8 scale; exp cannot overflow).
  3. y^T [65, Tq] accumulated over key tiles; row 64 is the denominator,
     bounced to DRAM, reloaded partition-major [128, 8] (so the slow DVE
     reciprocal costs ~8 free elements instead of 512), written back and
     broadcast across 64 partitions by 0-stride HWDGE DMAs. The final
     normalization multiplies run on GPSIMD, one chunk later, so no
     compute queue ever head-of-line blocks on this DMA chain.
  4. out partial [Tq 128, 1024] = y^T.T @ W_proj_shard, fp16.
  5. Tensor-engine saturation: stage-1 of chunk ch+1 and every ready
     projection tile are emitted as filler units INTO attention(ch)'s
     QK/exp/AV pipeline, so exp-chain waits never idle the PE (keeping
     its DVFS clock up). The startup loads are split per-C-slab across
     two HWDGE queues so the first matmul starts ~10us in, and the tail
     orders projection tiles so the last denominator chain is covered.
"""
import sys
import numpy as np

if "/opt/trn_rl_repo" not in sys.path:
    sys.path.insert(0, "/opt/trn_rl_repo")

B, T, C, H = 2, 2048, 1024, 16
D = C // H            # 64
NCORES = 8
HPC = H // NCORES     # 2 heads per core
BT = B * T            # 4096
QKV = 3 * HPC * D     # 384 per-core qkv width
NCH = BT // 512       # 8 T-chunks of 512
KT = T // 128         # 16 key tiles per batch
ICH = T // 512        # 4 query chunks per batch
VS = 160              # stride of V_aug tiles: [v_h0|1|pad|v_h1|1|pad]

_PROG = None


def _split_wide_waits(nc, max_waits=1):
    """walrus in this build accepts a single sync wait per instruction;
    Tile's tail drain aggregates one per engine/DMA lane. Split them."""
    import concourse.mybir as mybir
    ctr = 0
    for f in nc.m.functions:
        for bb in f.blocks:
            out = []
            for inst in bb.instructions:
                si = inst.sync_info
                if si is not None and si.on_wait and len(si.on_wait) > max_waits:
                    waits = list(si.on_wait)
                    chunks = [waits[i:i + max_waits]
                              for i in range(0, len(waits), max_waits)]
                    for ch in chunks[:-1]:
                        nop = mybir.InstDrain(
                            name=f"I-wsplit-{ctr}", ins=[], outs=[])
                        ctr += 1
                        nop.engine = inst.engine
                        nop.sync_info = mybir.SyncInfo(on_wait=ch, on_update=[])
                        out.append(nop)
                    inst.sync_info = mybir.SyncInfo(
                        on_wait=chunks[-1],
                        on_update=list(si.on_update) if si.on_update else [])
                out.append(inst)
            bb.instructions = out


def _build_program():
    import concourse.bass as bass
    import concourse.mybir as mybir
    import concourse.tile as tile

    f32 = mybir.dt.float32
    f16 = mybir.dt.float16
    AF = mybir.ActivationFunctionType
    ALU = mybir.AluOpType

    nc = bass.Bass()
    xt_d = nc.declare_dram_parameter("xT", [C, BT], f16, isOutput=False)
    wqkv_d = nc.declare_dram_parameter("wqkv", [C, QKV], f16, isOutput=False)
    wproj_d = nc.declare_dram_parameter("wproj", [HPC * D, C], f16,
                                        isOutput=False)
    out_d = nc.declare_dram_parameter("out", [BT, C], f16, isOutput=True)
    rs_d = nc.dram_tensor("rscratch", [4, T], f32)   # raw denominators
    rr_d = nc.dram_tensor("rrecip", [4, T], f32)     # their reciprocals

    with tile.TileContext(nc) as tc:
        with tc.tile_pool(name="const", bufs=1) as const, \
             tc.tile_pool(name="persist", bufs=1) as persist:
            w_h = const.tile([128, 8 * QKV], f16)
            wp_h = const.tile([128, C], f16)
            trimask = const.tile([128, 128], f16)  # keep col >= partition

            # combined qkv^T buffer: q at 0, k at BT, v^T at 2*BT
            qkv_sb = persist.tile([128, 3 * BT], f16)
            v_sb = persist.tile([128, 32 * VS], f16)   # V_aug tiles
            y0 = persist.tile([128, T], f16)           # y^T per b
            y1 = persist.tile([128, T], f16)
            ys = [y0, y1]
            rbc0 = persist.tile([128, T], f32)         # broadcast denominators
            rbc1 = persist.tile([128, T], f32)
            rbcs = [rbc0, rbc1]
            sums_st = persist.tile([1, 4 * T], f32)  # per-pair denominators
            vv = v_sb.rearrange("p (j s) -> p j s", s=VS)

            with tc.tile_pool(name="wload", bufs=1) as wload, \
                 tc.tile_pool(name="xt", bufs=8) as xt_pool, \
                 tc.tile_pool(name="pp", bufs=6) as p_pool, \
                 tc.tile_pool(name="rq", bufs=2) as rq_pool, \
                 tc.tile_pool(name="osb", bufs=6) as o_pool, \
                 tc.tile_pool(name="mm_ps", bufs=2, space="PSUM") as mm_ps, \
                 tc.tile_pool(name="s_ps", bufs=2, space="PSUM") as s_ps, \
                 tc.tile_pool(name="y_ps", bufs=2, space="PSUM") as y_ps:
                xt_tiles = {}

                def emit_load(ch, split=False, engine=None):
                    # x^T arrives pre-transposed+fp16 from the host
                    eng = engine or nc.sync
                    xt_all = xt_pool.tile([128, 8, 512], f16, tag="xt",
                                          name=f"xta{ch}")
                    src = xt_d[:, ch * 512:(ch + 1) * 512].rearrange(
                        "(cb p) t -> p cb t", p=128)
                    if split:
                        # per-cb pieces: the first stage-1 matmul only
                        # needs cb=0's slice, which lands in ~1us
                        for cb in range(8):
                            eng.dma_start(out=xt_all[:, cb, :],
                                          in_=src[:, cb, :])
                    else:
                        eng.dma_start(out=xt_all, in_=src)
                    xt_tiles[ch] = xt_all

                # startup: xt0 pieces stream on the sync queue while the
                # weight slabs + xt1 flow on the scalar HWDGE queue, so the
                # first matmuls and the first v-xbar are gated only by
                # their own slices
                w_src = wqkv_d.rearrange("(c p) m -> p c m", p=128)
                w_dst = w_h.rearrange("p (c m) -> p c m", c=8)
                for cb in range(8):
                    nc.scalar.dma_start(out=w_dst[:, cb, :],
                                        in_=w_src[:, cb, :])
                emit_load(0, split=True)
                emit_load(1, engine=nc.scalar)
                nc.scalar.dma_start(out=wp_h[:, :], in_=wproj_d[:, :])
                ones32 = wload.tile([128, 32], f16)
                nc.vector.memset(ones32[:, :], 1.0)
                nc.vector.tensor_copy(vv[:, :, 64], ones32[:, :])
                nc.vector.tensor_copy(vv[:, :, 144], ones32[:, :])
                # constant lower-triangular mask: keep iff col >= partition
                nc.vector.memset(trimask[:, :], 1.0)
                nc.gpsimd.affine_select(
                    out=trimask[:, :], in_=trimask[:, :],
                    compare_op=ALU.is_ge, fill=0.0,
                    base=0, channel_multiplier=-1, pattern=[[1, 128]],
                )

                def stage1_units(ch):
                    # q first (gates the first QK of this chunk), then v
                    # (feeds the xbar for AV), then k (only needed by the
                    # diagonal groups at the END of this chunk's attention).
                    # Each unit is one PE matmul; the last of an m-group
                    # carries the PSUM evacuation (+ v xbar transposes).
                    state = {}

                    def unit(m, cb):
                        def run():
                            if cb == 0:
                                state['acc'] = mm_ps.tile(
                                    [128, 512], f32, tag="mm",
                                    name=f"acc{ch}_{m}")
                            acc = state['acc']
                            nc.tensor.matmul(
                                acc[:, :],
                                w_h[:, cb * QKV + m * 128:
                                    cb * QKV + (m + 1) * 128],
                                xt_tiles[ch][:, cb, :],
                                start=(cb == 0), stop=(cb == 7),
                            )
                            if cb != 7:
                                return
                            nc.vector.tensor_copy(
                                qkv_sb[:, m * BT + ch * 512:
                                       m * BT + (ch + 1) * 512],
                                acc[:, :])
                            if m == 2:
                                for h in range(HPC):
                                    nc.sync.dma_start(
                                        out=vv[:, ch * 4:(ch + 1) * 4,
                                               h * 80:h * 80 + 64],
                                        in_=qkv_sb[h * 64:(h + 1) * 64,
                                                   2 * BT + ch * 512:
                                                   2 * BT + (ch + 1) * 512],
                                        transpose=True)
                            if m == 1:  # last m-group: chunk done
                                xt_tiles.pop(ch)
                        return run

                    return [unit(m, cb) for m in (0, 2, 1)
                            for cb in range(8)]

                def emit_attention(b, i, filler=None):
                    ng2 = 2 * (i + 1)          # groups of 2 key tiles
                    njt = 4 * (i + 1)
                    qs = b * T + i * 512
                    pt_h = {}
                    yps = [y_ps.tile([65, 512], f32, tag="y",
                                     name=f"yps{b}_{i}_{h}")
                           for h in range(HPC)]

                    def qoff_of(j):
                        return 128 * (j - 4 * i) if j >= 4 * i else 0

                    def emit_qk(gg, h):
                        st = s_ps.tile([128, 2, 512], f32, tag="s",
                                       name=f"st{b}_{i}_{gg}_{h}")
                        for u in range(2):
                            j = 2 * gg + u
                            qo = qoff_of(j)
                            nc.tensor.matmul(
                                st[:, u, qo:512],
                                qkv_sb[h * 64:(h + 1) * 64,
                                       BT + b * T + j * 128:
                                       BT + b * T + (j + 1) * 128],
                                qkv_sb[h * 64:(h + 1) * 64,
                                       qs + qo:qs + 512],
                                start=True, stop=True,
                            )
                        pt = p_pool.tile([128, 2, 512], f16, tag="p",
                                         name=f"pt{b}_{i}_{gg}_{h}")
                        if 2 * gg >= 4 * i:
                            # diagonal group: ragged exp + triangular mask
                            for u in range(2):
                                j = 2 * gg + u
                                qo = qoff_of(j)
                                nc.scalar.activation(
                                    pt[:, u, qo:512], st[:, u, qo:512],
                                    AF.Exp, scale=0.125)
                            for u in range(2):
                                j = 2 * gg + u
                                qo = qoff_of(j)
                                nc.vector.tensor_mul(
                                    pt[:, u, qo:qo + 128],
                                    pt[:, u, qo:qo + 128],
                                    trimask[:, :])
                        else:
                            nc.scalar.activation(
                                pt.rearrange("p u t -> p (u t)"),
                                st.rearrange("p u t -> p (u t)"),
                                AF.Exp, scale=0.125)
                        pt_h[(gg, h)] = pt

                    def emit_av(gg, h):
                        pt = pt_h.pop((gg, h))
                        for u in range(2):
                            j = 2 * gg + u
                            jg = b * KT + j
                            qo = qoff_of(j)
                            nc.tensor.matmul(
                                yps[h][0:65, qo:512],
                                v_sb[:, jg * VS + h * 80:
                                     jg * VS + h * 80 + 65],
                                pt[:, u, qo:512],
                                start=(j == 0), stop=(j == njt - 1),
                            )

                    # head-sequential pipeline over (gg, h) pairs
                    seq = [(gg, h) for gg in range(ng2) for h in range(HPC)]
                    W = 3
                    if filler is not None:
                        filler(0, len(seq) + 1)
                    for n, (gg, h) in enumerate(seq):
                        emit_qk(gg, h)
                        if filler is not None:
                            filler(n + 1, len(seq) + 1)
                        if n >= W:
                            emit_av(*seq[n - W])
                    for gg, h in seq[max(0, len(seq) - W):]:
                        emit_av(gg, h)

                    sl = slice(i * 512, (i + 1) * 512)
                    for h in range(HPC):
                        p = b * HPC + h
                        # split the y evacuations across engines so the
                        # next chunk's first AV (waiting on this PSUM pool)
                        # isn't gated on a backlogged DVE queue
                        if h == 0:
                            nc.scalar.copy(
                                ys[b][h * 64:(h + 1) * 64, sl],
                                yps[h][0:64, :])
                        else:
                            nc.vector.tensor_copy(
                                ys[b][h * 64:(h + 1) * 64, sl],
                                yps[h][0:64, :])
                        ssl = slice(p * T + i * 512, p * T + (i + 1) * 512)
                        nc.vector.tensor_copy(
                            sums_st[0:1, ssl], yps[h][64:65, :])
                        nc.sync.dma_start(out=rs_d[p:p + 1, sl],
                                          in_=sums_st[0:1, ssl])

                def emit_denom_chain(b, i):
                    # emitted one chunk later, so every hop is off the
                    # critical path; the final multiplies run on GPSIMD so
                    # neither the DVE nor scalar queue ever head-of-line
                    # blocks on this chain
                    sl = slice(i * 512, (i + 1) * 512)
                    # reciprocal in partition-major layout ([128, 8] costs
                    # ~8 free elems on DVE vs 512 for a row-major recip)
                    rq = rq_pool.tile([128, 2, 4], f32, tag="rq",
                                      name=f"rq{b}_{i}")
                    nc.sync.dma_start(
                        out=rq,
                        in_=rs_d[2 * b:2 * b + 2, sl].rearrange(
                            "r (p k) -> p r k", p=128))
                    nc.vector.reciprocal(rq[:, :, :], rq[:, :, :])
                    nc.sync.dma_start(
                        out=rr_d[2 * b:2 * b + 2, sl].rearrange(
                            "r (p k) -> p r k", p=128),
                        in_=rq)
                    for h in range(HPC):
                        p = b * HPC + h
                        nc.sync.dma_start(
                            out=rbcs[b][h * 64:(h + 1) * 64, sl],
                            in_=rr_d[p:p + 1, sl]
                            .partition_broadcast(64).squeeze(1),
                        )
                        nc.gpsimd.tensor_mul(
                            ys[b][h * 64:(h + 1) * 64, sl],
                            ys[b][h * 64:(h + 1) * 64, sl],
                            rbcs[b][h * 64:(h + 1) * 64, sl])

                o_tiles = {}

                def emit_proj_half(b, ts_, n, evac_engine):
                    if n == 0:
                        o_tiles[(b, ts_)] = o_pool.tile(
                            [128, C], f16, tag="o", name=f"osb{b}_{ts_}")
                    o_sb = o_tiles[(b, ts_)]
                    op = mm_ps.tile([128, 512], f32, tag="mm",
                                    name=f"ops{b}_{ts_}_{n}")
                    nc.tensor.matmul(
                        op[:, :],
                        ys[b][:, ts_ * 128:(ts_ + 1) * 128],
                        wp_h[:, n * 512:(n + 1) * 512],
                        start=True, stop=True,
                    )
                    if evac_engine == "both":
                        # tail mode: split across both engines to halve the
                        # PSUM-rotation latency (no exps compete there)
                        nc.scalar.copy(o_sb[:, n * 512:n * 512 + 256],
                                       op[:, 0:256])
                        nc.vector.tensor_copy(
                            o_sb[:, n * 512 + 256:(n + 1) * 512],
                            op[:, 256:512])
                    elif evac_engine == "act":
                        nc.scalar.copy(o_sb[:, n * 512:(n + 1) * 512],
                                       op[:, :])
                    else:
                        nc.vector.tensor_copy(
                            o_sb[:, n * 512:(n + 1) * 512], op[:, :])
                    if n == 1:
                        del o_tiles[(b, ts_)]
                        nc.sync.dma_start(
                            out=out_d[b * T + ts_ * 128:
                                      b * T + (ts_ + 1) * 128, :],
                            in_=o_sb[:, :])

                def proj_unit(b, ts_, n, eng=None):
                    def run():
                        emit_proj_half(
                            b, ts_, n,
                            eng or ("act" if (ts_ + n) % 2 else "dve"))
                    return run

                # PE filler-unit queue: stage-1 of the next chunk and ready
                # projection tiles, interleaved into attention emission so
                # the tensor engine never idles on the exp chain
                unit_q = []

                def take_units(k):
                    for _ in range(min(k, len(unit_q))):
                        unit_q.pop(0)()

                def filler(n, nseq):
                    slots = nseq - n
                    k = (len(unit_q) + slots - 1) // slots
                    take_units(k)

                # ---- emission schedule ----
                for u in stage1_units(0):
                    u()
                for ch in range(NCH):
                    b, i = ch // 4, ch % 4
                    if ch >= 1:
                        emit_denom_chain((ch - 1) // 4, (ch - 1) % 4)
                    if ch + 1 < NCH:
                        unit_q.extend(stage1_units(ch + 1))
                    # ready projection tiles as extra filler
                    if b == 1:
                        unit_q.extend(proj_unit(0, ts_, n)
                                      for ts_ in range(i * 4, i * 4 + 4)
                                      for n in range(2))
                    if ch == 6:
                        unit_q.extend(proj_unit(1, ts_, n)
                                      for ts_ in range(4)
                                      for n in range(2))
                    emit_attention(b, i, filler=filler)
                    take_units(len(unit_q))  # drain leftovers
                    # prefetch emitted last: with bufs=8 the xt pool never
                    # reuses a buffer, and consumers can't get their waits
                    # coarsened onto this later DMA
                    if ch + 2 < NCH:
                        emit_load(ch + 2)
                emit_denom_chain(1, 3)
                # tail: ts4..11 only need earlier chunks' normalization, so
                # they cover the exposed (1,3) denominator chain latency;
                # ts12..15 (which need it) come last
                for ts_ in range(4, KT):
                    for n in range(2):
                        emit_proj_half(1, ts_, n, "both")

    _split_wide_waits(nc)
    return nc


def _get_program():
    global _PROG
    if _PROG is None:
        _PROG = _build_program()
    return _PROG


def _make_in_maps(x, W_attn, W_proj):
    x = np.asarray(x, dtype=np.float32).reshape(BT, C)
    xT = np.ascontiguousarray(x.T.astype(np.float16))      # [C, BT] fp16
    W_attn = np.asarray(W_attn, dtype=np.float32)
    W_proj = np.asarray(W_proj, dtype=np.float32)
    in_maps = []
    for c in range(NCORES):
        lo = c * HPC * D
        hi = lo + HPC * D
        wqkv = np.ascontiguousarray(np.concatenate(
            [W_attn[:, lo:hi], W_attn[:, C + lo:C + hi],
             W_attn[:, 2 * C + lo:2 * C + hi]], axis=1).astype(np.float16))
        wproj = np.ascontiguousarray(W_proj[lo:hi, :].astype(np.float16))
        in_maps.append({"xT": xT, "wqkv": wqkv, "wproj": wproj})
    return in_maps


def kernel(x, W_attn, W_proj):
    from concourse.bass_utils import run_bass_kernel_spmd

    in_maps = _make_in_maps(x, W_attn, W_proj)
    nc = _get_program()
    res = run_bass_kernel_spmd(nc, in_maps, list(range(NCORES)))
    out = res.results[0]["out"].astype(np.float64)
    for c in range(1, NCORES):
        out += res.results[c]["out"]
    return out.astype(np.float32).reshape(B, T, C)


# revision 39
# speedup vs baseline: 1.0415x; 1.0415x over previous
"""Causal self-attention (B=2, T=2048, C=1024, H=16, D=64) on 8 trn2 NeuronCores.

Sharding: tensor-parallel over heads. Each core owns 2 heads:
  - W_attn columns for its heads (q/k/v slices)  -> per-core qkv width 384
  - W_proj rows for its heads                    -> per-core partial output
Attention is embarrassingly parallel over (B, head). Each core returns a
partial [B*T, C] output (y_local @ W_proj_shard) in fp16; the host sums
the 8 partials in float64 (the row-parallel unshard reduction).

Design notes (all matmul operands fp16, accumulation fp32 in PSUM):
  0. HOST pre-staging: x is cast to fp16 AND transposed on the host, so the
     device reads x^T [C, BT] with plain contiguous DMAs -- no on-device
     f32->f16 cast (was 34us DVE) and no xbar DMA-transposes of x (was 65us
     of DMA fabric). W_attn/W_proj are host-cast to fp16 too.
  1. qkvT [384, 4096] = W^T x^T accumulated over 8 C-tiles into PSUM,
     evacuated (DVE) to a combined fp16 qkv^T buffer. v^T is re-transposed
     by 2 small xbar DMAs/chunk into natural V_aug tiles
     [Tk 128, v_h0|1|pad|v_h1|1|pad] (stride 160, 32B-aligned dests); each
     head's stationary operand is a contiguous 65-column slice whose
     ones-column computes the softmax denominators for free.
  2. Scores TRANSPOSED: S^T [Tk 128, Tq 512] = k^T.T @ q^T (K=64), TWO
     key-tiles per 2-bank PSUM tile (double-buffered) so the QK->exp->AV
     chain pipelines. Causality exploited at 128 granularity: for diagonal
     key-tiles the QK/exp/AV all start at query column 128*(j-4i) (the
     block-triangular raggedness), and the 128x128 triangular blocks are
     masked by a DVE multiply with a constant lower-tri mask (no GPSIMD
     affine_select in the inner loop). Softmax max-subtraction is skipped
     (logits ~N(0,1) after the 1/8 scale; exp cannot overflow).
  3. y^T [65, Tq] accumulated over key tiles; row 64 is the denominator,
     bounced to DRAM, reloaded partition-major [128, 8] (so the slow DVE
     reciprocal costs ~8 free elements instead of 512), written back and
     broadcast across 64 partitions by 0-stride HWDGE DMAs. The final
     normalization multiplies run on GPSIMD, one chunk later, so no
     compute queue ever head-of-line blocks on this DMA chain.
  4. out partial [Tq 128, 1024] = y^T.T @ W_proj_shard, fp16.
  5. Tensor-engine saturation: stage-1 of chunk ch+1 and every ready
     projection tile are emitted as filler units INTO attention(ch)'s
     QK/exp/AV pipeline, so exp-chain waits never idle the PE (keeping
     its DVFS clock up). The startup loads are split per-C-slab across
     two HWDGE queues so the first matmul starts ~10us in, and the tail
     orders projection tiles so the last denominator chain is covered.
"""
import sys
import numpy as np

if "/opt/trn_rl_repo" not in sys.path:
    sys.path.insert(0, "/opt/trn_rl_repo")

B, T, C, H = 2, 2048, 1024, 16
D = C // H            # 64
NCORES = 8
HPC = H // NCORES     # 2 heads per core
BT = B * T            # 4096
QKV = 3 * HPC * D     # 384 per-core qkv width
NCH = BT // 512       # 8 T-chunks of 512
KT = T // 128         # 16 key tiles per batch
ICH = T // 512        # 4 query chunks per batch
VS = 160              # stride of V_aug tiles: [v_h0|1|pad|v_h1|1|pad]

_PROG = None


def _split_wide_waits(nc, max_waits=1):
    """walrus in this build accepts a single sync wait per instruction;
    Tile's tail drain aggregates one per engine/DMA lane. Split them."""
    import concourse.mybir as mybir
    ctr = 0
    for f in nc.m.functions:
        for bb in f.blocks:
            out = []
            for inst in bb.instructions:
                si = inst.sync_info
                if si is not None and si.on_wait and len(si.on_wait) > max_waits:
                    waits = list(si.on_wait)
                    chunks = [waits[i:i + max_waits]
                              for i in range(0, len(waits), max_waits)]
                    for ch in chunks[:-1]:
                        nop = mybir.InstDrain(
                            name=f"I-wsplit-{ctr}", ins=[], outs=[])
                        ctr += 1
                        nop.engine = inst.engine
                        nop.sync_info = mybir.SyncInfo(on_wait=ch, on_update=[])
                        out.append(nop)
                    inst.sync_info = mybir.SyncInfo(
                        on_wait=chunks[-1],
                        on_update=list(si.on_update) if si.on_update else [])
                out.append(inst)
            bb.instructions = out


def _build_program():
    import concourse.bass as bass
    import concourse.mybir as mybir
    import concourse.tile as tile

    f32 = mybir.dt.float32
    f16 = mybir.dt.float16
    AF = mybir.ActivationFunctionType
    ALU = mybir.AluOpType

    nc = bass.Bass()
    xt_d = nc.declare_dram_parameter("xT", [C, BT], f16, isOutput=False)
    wqkv_d = nc.declare_dram_parameter("wqkv", [C, QKV], f16, isOutput=False)
    wproj_d = nc.declare_dram_parameter("wproj", [HPC * D, C], f16,
                                        isOutput=False)
    out_d = nc.declare_dram_parameter("out", [BT, C], f16, isOutput=True)
    rs_d = nc.dram_tensor("rscratch", [4, T], f32)   # raw denominators
    rr_d = nc.dram_tensor("rrecip", [4, T], f32)     # their reciprocals

    with tile.TileContext(nc) as tc:
        with tc.tile_pool(name="const", bufs=1) as const, \
             tc.tile_pool(name="persist", bufs=1) as persist:
            w_h = const.tile([128, 8 * QKV], f16)
            wp_h = const.tile([128, C], f16)
            trimask = const.tile([128, 128], f16)  # keep col >= partition

            # combined qkv^T buffer: q at 0, k at BT, v^T at 2*BT
            qkv_sb = persist.tile([128, 3 * BT], f16)
            v_sb = persist.tile([128, 32 * VS], f16)   # V_aug tiles
            y0 = persist.tile([128, T], f16)           # y^T per b
            y1 = persist.tile([128, T], f16)
            ys = [y0, y1]
            rbc0 = persist.tile([128, T], f32)         # broadcast denominators
            rbc1 = persist.tile([128, T], f32)
            rbcs = [rbc0, rbc1]
            sums_st = persist.tile([1, 4 * T], f32)  # per-pair denominators
            vv = v_sb.rearrange("p (j s) -> p j s", s=VS)

            with tc.tile_pool(name="wload", bufs=1) as wload, \
                 tc.tile_pool(name="xt", bufs=8) as xt_pool, \
                 tc.tile_pool(name="pp", bufs=6) as p_pool, \
                 tc.tile_pool(name="rq", bufs=2) as rq_pool, \
                 tc.tile_pool(name="osb", bufs=6) as o_pool, \
                 tc.tile_pool(name="mm_ps", bufs=2, space="PSUM") as mm_ps, \
                 tc.tile_pool(name="s_ps", bufs=2, space="PSUM") as s_ps, \
                 tc.tile_pool(name="y_ps", bufs=2, space="PSUM") as y_ps:
                xt_tiles = {}

                def emit_load(ch, split=False, engine=None):
                    # x^T arrives pre-transposed+fp16 from the host
                    eng = engine or nc.sync
                    xt_all = xt_pool.tile([128, 8, 512], f16, tag="xt",
                                          name=f"xta{ch}")
                    src = xt_d[:, ch * 512:(ch + 1) * 512].rearrange(
                        "(cb p) t -> p cb t", p=128)
                    if split:  # halves so the first matmul starts earlier
                        eng.dma_start(out=xt_all[:, 0:4, :],
                                      in_=src[:, 0:4, :])
                        eng.dma_start(out=xt_all[:, 4:8, :],
                                      in_=src[:, 4:8, :])
                    else:
                        eng.dma_start(out=xt_all, in_=src)
                    xt_tiles[ch] = xt_all

                # startup: xt0 halves stream on the sync queue while the
                # weights flow on the scalar HWDGE queue in parallel
                emit_load(0, split=True)
                nc.scalar.dma_start(
                    out=w_h.rearrange("p (c m) -> p c m", c=8),
                    in_=wqkv_d.rearrange("(c p) m -> p c m", p=128),
                )
                emit_load(1)
                nc.scalar.dma_start(out=wp_h[:, :], in_=wproj_d[:, :])
                ones32 = wload.tile([128, 32], f16)
                nc.vector.memset(ones32[:, :], 1.0)
                nc.vector.tensor_copy(vv[:, :, 64], ones32[:, :])
                nc.vector.tensor_copy(vv[:, :, 144], ones32[:, :])
                # constant lower-triangular mask: keep iff col >= partition
                nc.vector.memset(trimask[:, :], 1.0)
                nc.gpsimd.affine_select(
                    out=trimask[:, :], in_=trimask[:, :],
                    compare_op=ALU.is_ge, fill=0.0,
                    base=0, channel_multiplier=-1, pattern=[[1, 128]],
                )

                def stage1_units(ch):
                    # q first (gates the first QK of this chunk), then v
                    # (feeds the xbar for AV), then k (only needed by the
                    # diagonal groups at the END of this chunk's attention).
                    # Each unit is one PE matmul; the last of an m-group
                    # carries the PSUM evacuation (+ v xbar transposes).
                    state = {}

                    def unit(m, cb):
                        def run():
                            if cb == 0:
                                state['acc'] = mm_ps.tile(
                                    [128, 512], f32, tag="mm",
                                    name=f"acc{ch}_{m}")
                            acc = state['acc']
                            nc.tensor.matmul(
                                acc[:, :],
                                w_h[:, cb * QKV + m * 128:
                                    cb * QKV + (m + 1) * 128],
                                xt_tiles[ch][:, cb, :],
                                start=(cb == 0), stop=(cb == 7),
                            )
                            if cb != 7:
                                return
                            nc.vector.tensor_copy(
                                qkv_sb[:, m * BT + ch * 512:
                                       m * BT + (ch + 1) * 512],
                                acc[:, :])
                            if m == 2:
                                for h in range(HPC):
                                    nc.sync.dma_start(
                                        out=vv[:, ch * 4:(ch + 1) * 4,
                                               h * 80:h * 80 + 64],
                                        in_=qkv_sb[h * 64:(h + 1) * 64,
                                                   2 * BT + ch * 512:
                                                   2 * BT + (ch + 1) * 512],
                                        transpose=True)
                            if m == 1:  # last m-group: chunk done
                                xt_tiles.pop(ch)
                        return run

                    return [unit(m, cb) for m in (0, 2, 1)
                            for cb in range(8)]

                def emit_attention(b, i, filler=None):
                    ng2 = 2 * (i + 1)          # groups of 2 key tiles
                    njt = 4 * (i + 1)
                    qs = b * T + i * 512
                    pt_h = {}
                    yps = [y_ps.tile([65, 512], f32, tag="y",
                                     name=f"yps{b}_{i}_{h}")
                           for h in range(HPC)]

                    def qoff_of(j):
                        return 128 * (j - 4 * i) if j >= 4 * i else 0

                    def emit_qk(gg, h):
                        st = s_ps.tile([128, 2, 512], f32, tag="s",
                                       name=f"st{b}_{i}_{gg}_{h}")
                        for u in range(2):
                            j = 2 * gg + u
                            qo = qoff_of(j)
                            nc.tensor.matmul(
                                st[:, u, qo:512],
                                qkv_sb[h * 64:(h + 1) * 64,
                                       BT + b * T + j * 128:
                                       BT + b * T + (j + 1) * 128],
                                qkv_sb[h * 64:(h + 1) * 64,
                                       qs + qo:qs + 512],
                                start=True, stop=True,
                            )
                        pt = p_pool.tile([128, 2, 512], f16, tag="p",
                                         name=f"pt{b}_{i}_{gg}_{h}")
                        if 2 * gg >= 4 * i:
                            # diagonal group: ragged exp + triangular mask
                            for u in range(2):
                                j = 2 * gg + u
                                qo = qoff_of(j)
                                nc.scalar.activation(
                                    pt[:, u, qo:512], st[:, u, qo:512],
                                    AF.Exp, scale=0.125)
                            for u in range(2):
                                j = 2 * gg + u
                                qo = qoff_of(j)
                                nc.vector.tensor_mul(
                                    pt[:, u, qo:qo + 128],
                                    pt[:, u, qo:qo + 128],
                                    trimask[:, :])
                        else:
                            nc.scalar.activation(
                                pt.rearrange("p u t -> p (u t)"),
                                st.rearrange("p u t -> p (u t)"),
                                AF.Exp, scale=0.125)
                        pt_h[(gg, h)] = pt

                    def emit_av(gg, h):
                        pt = pt_h.pop((gg, h))
                        for u in range(2):
                            j = 2 * gg + u
                            jg = b * KT + j
                            qo = qoff_of(j)
                            nc.tensor.matmul(
                                yps[h][0:65, qo:512],
                                v_sb[:, jg * VS + h * 80:
                                     jg * VS + h * 80 + 65],
                                pt[:, u, qo:512],
                                start=(j == 0), stop=(j == njt - 1),
                            )

                    # head-sequential pipeline over (gg, h) pairs
                    seq = [(gg, h) for gg in range(ng2) for h in range(HPC)]
                    W = 3
                    if filler is not None:
                        filler(0, len(seq) + 1)
                    for n, (gg, h) in enumerate(seq):
                        emit_qk(gg, h)
                        if filler is not None:
                            filler(n + 1, len(seq) + 1)
                        if n >= W:
                            emit_av(*seq[n - W])
                    for gg, h in seq[max(0, len(seq) - W):]:
                        emit_av(gg, h)

                    sl = slice(i * 512, (i + 1) * 512)
                    for h in range(HPC):
                        p = b * HPC + h
                        nc.vector.tensor_copy(
                            ys[b][h * 64:(h + 1) * 64, sl],
                            yps[h][0:64, :])
                        ssl = slice(p * T + i * 512, p * T + (i + 1) * 512)
                        nc.vector.tensor_copy(
                            sums_st[0:1, ssl], yps[h][64:65, :])
                        nc.sync.dma_start(out=rs_d[p:p + 1, sl],
                                          in_=sums_st[0:1, ssl])

                def emit_denom_chain(b, i):
                    # emitted one chunk later, so every hop is off the
                    # critical path; the final multiplies run on GPSIMD so
                    # neither the DVE nor scalar queue ever head-of-line
                    # blocks on this chain
                    sl = slice(i * 512, (i + 1) * 512)
                    # reciprocal in partition-major layout ([128, 8] costs
                    # ~8 free elems on DVE vs 512 for a row-major recip)
                    rq = rq_pool.tile([128, 2, 4], f32, tag="rq",
                                      name=f"rq{b}_{i}")
                    nc.sync.dma_start(
                        out=rq,
                        in_=rs_d[2 * b:2 * b + 2, sl].rearrange(
                            "r (p k) -> p r k", p=128))
                    nc.vector.reciprocal(rq[:, :, :], rq[:, :, :])
                    nc.sync.dma_start(
                        out=rr_d[2 * b:2 * b + 2, sl].rearrange(
                            "r (p k) -> p r k", p=128),
                        in_=rq)
                    for h in range(HPC):
                        p = b * HPC + h
                        nc.sync.dma_start(
                            out=rbcs[b][h * 64:(h + 1) * 64, sl],
                            in_=rr_d[p:p + 1, sl]
                            .partition_broadcast(64).squeeze(1),
                        )
                        nc.gpsimd.tensor_mul(
                            ys[b][h * 64:(h + 1) * 64, sl],
                            ys[b][h * 64:(h + 1) * 64, sl],
                            rbcs[b][h * 64:(h + 1) * 64, sl])

                o_tiles = {}

                def emit_proj_half(b, ts_, n, evac_engine):
                    if n == 0:
                        o_tiles[(b, ts_)] = o_pool.tile(
                            [128, C], f16, tag="o", name=f"osb{b}_{ts_}")
                    o_sb = o_tiles[(b, ts_)]
                    op = mm_ps.tile([128, 512], f32, tag="mm",
                                    name=f"ops{b}_{ts_}_{n}")
                    nc.tensor.matmul(
                        op[:, :],
                        ys[b][:, ts_ * 128:(ts_ + 1) * 128],
                        wp_h[:, n * 512:(n + 1) * 512],
                        start=True, stop=True,
                    )
                    if evac_engine == "both":
                        # tail mode: split across both engines to halve the
                        # PSUM-rotation latency (no exps compete there)
                        nc.scalar.copy(o_sb[:, n * 512:n * 512 + 256],
                                       op[:, 0:256])
                        nc.vector.tensor_copy(
                            o_sb[:, n * 512 + 256:(n + 1) * 512],
                            op[:, 256:512])
                    elif evac_engine == "act":
                        nc.scalar.copy(o_sb[:, n * 512:(n + 1) * 512],
                                       op[:, :])
                    else:
                        nc.vector.tensor_copy(
                            o_sb[:, n * 512:(n + 1) * 512], op[:, :])
                    if n == 1:
                        del o_tiles[(b, ts_)]
                        nc.sync.dma_start(
                            out=out_d[b * T + ts_ * 128:
                                      b * T + (ts_ + 1) * 128, :],
                            in_=o_sb[:, :])

                def proj_unit(b, ts_, n, eng=None):
                    def run():
                        emit_proj_half(
                            b, ts_, n,
                            eng or ("act" if (ts_ + n) % 2 else "dve"))
                    return run

                # PE filler-unit queue: stage-1 of the next chunk and ready
                # projection tiles, interleaved into attention emission so
                # the tensor engine never idles on the exp chain
                unit_q = []

                def take_units(k):
                    for _ in range(min(k, len(unit_q))):
                        unit_q.pop(0)()

                def filler(n, nseq):
                    slots = nseq - n
                    k = (len(unit_q) + slots - 1) // slots
                    take_units(k)

                # ---- emission schedule ----
                for u in stage1_units(0):
                    u()
                for ch in range(NCH):
                    b, i = ch // 4, ch % 4
                    if ch >= 1:
                        emit_denom_chain((ch - 1) // 4, (ch - 1) % 4)
                    if ch + 2 < NCH:
                        emit_load(ch + 2)
                    if ch + 1 < NCH:
                        unit_q.extend(stage1_units(ch + 1))
                    # ready projection tiles as extra filler
                    if b == 1:
                        unit_q.extend(proj_unit(0, ts_, n)
                                      for ts_ in range(i * 4, i * 4 + 4)
                                      for n in range(2))
                    if ch == 6:
                        unit_q.extend(proj_unit(1, ts_, n)
                                      for ts_ in range(4)
                                      for n in range(2))
                    emit_attention(b, i, filler=filler)
                    take_units(len(unit_q))  # drain leftovers
                emit_denom_chain(1, 3)
                # tail: ts4..11 only need earlier chunks' normalization, so
                # they cover the exposed (1,3) denominator chain latency;
                # ts12..15 (which need it) come last
                for ts_ in range(4, KT):
                    for n in range(2):
                        emit_proj_half(1, ts_, n, "both")

    _split_wide_waits(nc)
    return nc


def _get_program():
    global _PROG
    if _PROG is None:
        _PROG = _build_program()
    return _PROG


def _make_in_maps(x, W_attn, W_proj):
    x = np.asarray(x, dtype=np.float32).reshape(BT, C)
    xT = np.ascontiguousarray(x.T.astype(np.float16))      # [C, BT] fp16
    W_attn = np.asarray(W_attn, dtype=np.float32)
    W_proj = np.asarray(W_proj, dtype=np.float32)
    in_maps = []
    for c in range(NCORES):
        lo = c * HPC * D
        hi = lo + HPC * D
        wqkv = np.ascontiguousarray(np.concatenate(
            [W_attn[:, lo:hi], W_attn[:, C + lo:C + hi],
             W_attn[:, 2 * C + lo:2 * C + hi]], axis=1).astype(np.float16))
        wproj = np.ascontiguousarray(W_proj[lo:hi, :].astype(np.float16))
        in_maps.append({"xT": xT, "wqkv": wqkv, "wproj": wproj})
    return in_maps


def kernel(x, W_attn, W_proj):
    from concourse.bass_utils import run_bass_kernel_spmd

    in_maps = _make_in_maps(x, W_attn, W_proj)
    nc = _get_program()
    res = run_bass_kernel_spmd(nc, in_maps, list(range(NCORES)))
    out = res.results[0]["out"].astype(np.float64)
    for c in range(1, NCORES):
        out += res.results[c]["out"]
    return out.astype(np.float32).reshape(B, T, C)
